# revision 9
# baseline (speedup 1.0000x reference)
"""BCMGOOLSTM on 8 TRN2 NeuronCores — V11: col-tiled concurrent gate matmuls.

Strategy (hardcoded for T=1500, B=16, D=512, L=P=512, G=2048, 8 cores):
  - Shard batch: core i handles b in {2i, 2i+1} (B_loc=2).
  - Host prep: reconstruct block-circulant weights, fuse output projection:
        u_t = h_{t-1} @ Wc + (x_t @ WihT + bias),  Wc = wym_w.T @ W_hh.T
    gate blocks permuted to [f, c, i, o]; the c block of WihT/bias/Wc is
    pre-scaled by 2 so tanh(x) = 2*sigmoid(2x) - 1 folds into one fused
    sigmoid over all four gates.
  - Scan (per step): the 4 gate matmuls share the stationary h and run
    CONCURRENTLY in 4 column tiles (128x32 PE tiling) into one PSUM bank
    at partition strips 0/32/64/96.  w_out is injected via contract-padded
    identity matmuls (K=128, rows 2..127 zero) in the same tiling mode.
    One sigmoid activation covers all 4 gates; PE transposes (32x128 row
    tiles) return gates to [L-on-partitions, B]; cell update on VectorE
    with the tanh correction 2*sig-1 fused via tensor_scalar.
  - Phase 1 (w_out) and phase 3 (output projection) are plain matmuls.

Self-contained (includes the walrus single-sync-wait workaround).
"""

import os
import shutil

import numpy as np
import ml_dtypes

# The neuronxcc compile cache keys on an HLO hash that does not cover the
# bass kernel embedded in the custom-call payload, so different kernel
# versions with the same I/O interface collide on one cache entry.  Clear
# it once at import so this file always compiles (and then caches) its own
# NEFF instead of silently reusing a stale one.
shutil.rmtree(
    os.path.expanduser("~/.neuron-compile-cache"), ignore_errors=True
)

# ---------------------------------------------------------------------------
# Problem constants (hardcoded per spec)
# ---------------------------------------------------------------------------
T, B, D = 1500, 16, 512
L = 512
P_DIM = 512
G = 4 * L          # 2048
NCORES = 8
BLOC = B // NCORES  # 2
S_CHUNK = 8         # scan w_out chunk (steps per DMA)

_GATE_PERM = np.concatenate([
    np.arange(0, 512),        # f
    np.arange(1536, 2048),    # c
    np.arange(512, 1024),     # i
    np.arange(1024, 1536),    # o
]).astype(np.int64)

_BUILT = {}


# ---------------------------------------------------------------------------
# Walrus workaround: at most ONE semaphore wait per instruction
# ---------------------------------------------------------------------------
def _apply_tile_patches():
    import concourse.mybir as mybir
    import concourse.tile as tile_mod
    from concourse.vector_clock import ScopedClock

    def _drain_and_barrier(self, tick_clock, wait_clock):
        nc = self.nc
        drain_inst = nc.sync.drain()
        wait_clock.add_sem_waits(
            drain_inst.ins, ScopedClock({None: tick_clock.global_clock})
        )
        nc.all_engine_barrier()
        assert self.sems is not None
        popped = nc._tile_sem_poison_stack.pop()
        assert popped is self._sem_poison
        nc.clear_and_free_semaphores(list(self.sems.allocated().values()))
        nc.all_engine_barrier()

    tile_mod.TileContext._drain_and_barrier = _drain_and_barrier


def _fix_excess_waits(nc, max_waits=1):
    import concourse.mybir as mybir

    counter = 0
    for f in nc.m.functions:
        for blk in f.blocks:
            insts = list(blk.instructions)
            out = []
            changed = False
            for inst in insts:
                si = inst.sync_info
                if si is not None and len(si.on_wait) > max_waits:
                    waits = list(si.on_wait)
                    excess, keep = waits[:-max_waits], waits[-max_waits:]
                    for w in excess:
                        nop = mybir.InstNoOp(
                            name=f"waitspill-{counter}", ins=[], outs=[]
                        )
                        counter += 1
                        nop.engine = inst.engine
                        nop.sync_info = mybir.SyncInfo(on_wait=[w], on_update=[])
                        out.append(nop)
                    inst.sync_info = mybir.SyncInfo(
                        on_wait=keep, on_update=list(si.on_update)
                    )
                    changed = True
                out.append(inst)
            if changed:
                blk.instructions = out
    return counter


# ---------------------------------------------------------------------------
# Device kernel builder
# ---------------------------------------------------------------------------
def _build(t_steps=T, fix_waits=True, split_k3=True):
    import concourse.bass as bass
    import concourse.mybir as mybir
    from concourse.tile import TileContext, add_dep_helper

    F32 = mybir.dt.float32
    BF16 = mybir.dt.bfloat16
    Sigmoid = mybir.ActivationFunctionType.Sigmoid
    Tanh = mybir.ActivationFunctionType.Tanh
    Mult = mybir.AluOpType.mult
    Add = mybir.AluOpType.add

    tb = t_steps * BLOC

    nc = bass.Bass(trn_type="TRN2")
    xT_d = nc.declare_dram_parameter("xT", [D, tb], F32, isOutput=False)
    wihT_d = nc.declare_dram_parameter("wihT", [D, G], F32, isOutput=False)
    wc_d = nc.declare_dram_parameter("wc", [L, G], F32, isOutput=False)
    wymT_d = nc.declare_dram_parameter("wymT", [L, P_DIM], F32, isOutput=False)
    bias_d = nc.declare_dram_parameter("bias", [1, G], F32, isOutput=False)
    ident_d = nc.declare_dram_parameter("ident", [BLOC, BLOC], F32, isOutput=False)
    identt_d = nc.declare_dram_parameter("identt", [128, BLOC], F32, isOutput=False)
    ysT_d = nc.declare_dram_parameter("ysT", [P_DIM, tb], F32, isOutput=True)

    n_mtiles = (tb + 127) // 128

    with TileContext(nc) as tc:
        with tc.tile_pool(name="persist", bufs=1) as pp, \
             tc.tile_pool(name="dram", bufs=1, space="DRAM") as dp:

            # persistent SBUF tensors
            xT_sb = pp.tile([128, 4, tb], BF16)
            wihT_sb = pp.tile([128, 4, G], BF16)
            wc_sb = pp.tile([128, 4, G], BF16)
            wymT_sb = pp.tile([128, 4, P_DIM], BF16)
            bias_sb = pp.tile([1, G], BF16)
            ones_sb = pp.tile([1, 128], BF16)
            ident = pp.tile([BLOC, BLOC], BF16)
            identt = pp.tile([128, BLOC], BF16)
            ht_hist = pp.tile([128, 4, t_steps + 1, BLOC], BF16)

            nc.gpsimd.dma_start(xT_sb[:], xT_d.rearrange("(k p) n -> p k n", p=128))
            nc.gpsimd.dma_start(wihT_sb[:], wihT_d.rearrange("(k p) g -> p k g", p=128))
            nc.gpsimd.dma_start(wc_sb[:], wc_d.rearrange("(k p) g -> p k g", p=128))
            nc.gpsimd.dma_start(wymT_sb[:], wymT_d.rearrange("(k p) g -> p k g", p=128))
            nc.gpsimd.dma_start(bias_sb[:], bias_d[:])
            nc.gpsimd.dma_start(ident[:], ident_d[:])
            nc.gpsimd.dma_start(identt[:], identt_d[:])
            nc.vector.memset(ones_sb[:], 1.0)
            nc.vector.memset(ht_hist[:, :, 0, :], 0.0)

            wout_i = dp.tile([t_steps * BLOC, G], BF16)

            # ---------------- phase 1: w_out ----------------
            p1_dmas = []
            with tc.tile_pool(name="p1sb", bufs=3) as p1, \
                 tc.tile_pool(name="p1ps", bufs=3, space="PSUM") as p1p:
                for m in range(n_mtiles):
                    rows = min(128, tb - m * 128)
                    wtile = p1.tile([128, G], BF16, tag="wtile")
                    for nchunk in range(4):
                        pw = p1p.tile([128, 512], F32, tag="pw")
                        for k in range(4):
                            nc.tensor.matmul(
                                pw[:rows],
                                xT_sb[:, k, m * 128 : m * 128 + rows],
                                wihT_sb[:, k, nchunk * 512 : (nchunk + 1) * 512],
                                start=(k == 0),
                                stop=False,
                            )
                        nc.tensor.matmul(
                            pw[:rows],
                            ones_sb[:, :rows],
                            bias_sb[:, nchunk * 512 : (nchunk + 1) * 512],
                            start=False,
                            stop=True,
                        )
                        nc.vector.tensor_copy(
                            wtile[:rows, nchunk * 512 : (nchunk + 1) * 512],
                            pw[:rows],
                        )
                    p1_dmas.append(
                        nc.sync.dma_start(
                            wout_i[m * 128 : m * 128 + rows, :],
                            wtile[:rows],
                        )
                    )

            # ---------------- phase 2: scan (V11) ----------------
            # Gate strips: f@0, c@32, i@64, o@96 (perm block order f,c,i,o).
            with tc.tile_pool(name="wop", bufs=2) as wp, \
                 tc.tile_pool(name="gsb", bufs=2) as gp, \
                 tc.tile_pool(name="state", bufs=1) as stp, \
                 tc.tile_pool(name="gpsP", bufs=1, space="PSUM") as pgp, \
                 tc.tile_pool(name="pts", bufs=1, space="PSUM") as ptp:

                ctT_a = stp.tile([128, 4, BLOC], F32)
                ctT_b = stp.tile([128, 4, BLOC], F32)
                nc.vector.memset(ctT_a[:], 0.0)
                nc.vector.memset(ctT_b[:], 0.0)
                cts = [ctT_a, ctT_b]
                v3 = lambda ap: ap.rearrange("p (k b) -> p k b", k=4)

                # two persistent gate psum banks; the fused sigmoid reads the
                # gap partitions between strips, so initialize them once
                pg_a = pgp.tile([128, 512], F32)
                pg_b = pgp.tile([128, 512], F32)
                nc.vector.memset(pg_a[:], 0.0)
                nc.vector.memset(pg_b[:], 0.0)
                pgs = [pg_a, pg_b]

                def dma_chunk(cidx):
                    woc = wp.tile([BLOC, S_CHUNK * G], BF16, tag="wo")
                    n_here = min(S_CHUNK, t_steps - cidx * S_CHUNK)
                    rd = nc.sync.dma_start(
                        woc[:, : n_here * G].rearrange("b (t g) -> b t g", g=G),
                        wout_i[
                            cidx * S_CHUNK * BLOC : (cidx * S_CHUNK + n_here) * BLOC, :
                        ].rearrange("(t b) g -> b t g", b=BLOC),
                    )
                    # RAW through DRAM isn't tile-tracked: order the chunk
                    # read after the phase-1 write that produced it.
                    m = (cidx * S_CHUNK * BLOC) // 128
                    m2 = ((cidx * S_CHUNK + n_here) * BLOC - 1) // 128
                    add_dep_helper(rd.ins, p1_dmas[m].ins, reason="wout RAW")
                    if m2 != m and m2 < len(p1_dmas):
                        add_dep_helper(rd.ins, p1_dmas[m2].ins, reason="wout RAW2")
                    return woc

                def emit_k2(tt, woc):
                    s = tt % S_CHUNK
                    pg = pgs[tt % 2]
                    for g in range(4):
                        nc.tensor.matmul(
                            pg[32 * g : 32 * g + BLOC, :],
                            ident[:],
                            woc[:, s * G + g * 512 : s * G + (g + 1) * 512],
                            start=True,
                            stop=False,
                            tile_position=(0, 32 * g),
                        )
                    return pg

                wo = dma_chunk(0)
                pg = emit_k2(0, wo)

                for t in range(t_steps):
                    ctT_prev, ctT_new = cts[t % 2], cts[(t + 1) % 2]

                    # 4 concurrent col-tiled MMs per contract chunk; halves
                    # (k=0,1 | k=2,3) depend on the matching ht half of t-1.
                    # k=3 is split by column halves with piecewise stops so
                    # the H=0 sigmoid can fire before the H=1 columns finish
                    # (stop is sim-bookkeeping only, a no-op on hardware).
                    for k in range(3):
                        for g in range(4):
                            nc.tensor.matmul(
                                pg[32 * g : 32 * g + BLOC, :],
                                ht_hist[:, k, t, :],
                                wc_sb[:, k, g * 512 : (g + 1) * 512],
                                start=False,
                                stop=False,
                                tile_position=(0, 32 * g),
                            )
                    if split_k3:
                        for H in range(2):
                            cs = 256 * H
                            for g in range(4):
                                nc.tensor.matmul(
                                    pg[32 * g : 32 * g + BLOC, cs : cs + 256],
                                    ht_hist[:, 3, t, :],
                                    wc_sb[:, 3, g * 512 + cs : g * 512 + cs + 256],
                                    start=False,
                                    stop=True,
                                    tile_position=(0, 32 * g),
                                    skip_group_check=True,
                                )
                    else:
                        for g in range(4):
                            nc.tensor.matmul(
                                pg[32 * g : 32 * g + BLOC, :],
                                ht_hist[:, 3, t, :],
                                wc_sb[:, 3, g * 512 : (g + 1) * 512],
                                start=False,
                                stop=True,
                                tile_position=(0, 32 * g),
                            )

                    # K2s for t+1 fill the PE gap while ScalarE works
                    if t + 1 < t_steps:
                        if (t + 1) % S_CHUNK == 0:
                            wo = dma_chunk((t + 1) // S_CHUNK)
                        pg_next = emit_k2(t + 1, wo)

                    # pipelined by column halves: sigmoid -> transposes ->
                    # cell chain -> (next step's matching gate MMs)
                    sig = gp.tile([128, 512], BF16, tag="sig")
                    pts = []
                    for g in range(4):
                        ptile = ptp.tile(
                            [128, 1024], BF16, tag=f"pt{g}", name=f"pt{g}_{t}"
                        )
                        pts.append(ptile)
                    vph = lambda ptile, H: v3(ptile[:, 0:8])[:, 2 * H : 2 * H + 2, :]

                    for H in range(2):
                        cs = 256 * H
                        nc.scalar.activation(
                            sig[:98, cs : cs + 256],
                            pg[:98, cs : cs + 256],
                            Sigmoid,
                        )
                        for g in range(4):
                            for k in (2 * H, 2 * H + 1):
                                nc.tensor.transpose(
                                    pts[g][:, 2 * k : 2 * k + 2],
                                    sig[32 * g : 32 * g + BLOC, 128 * k : 128 * (k + 1)],
                                    identt[32 * g : 32 * g + BLOC, :],
                                    tile_position=(32 * g, 0),
                                )
                        hs = slice(2 * H, 2 * H + 2)
                        tanh_c = gp.tile([128, 2, BLOC], BF16, tag=f"tanh_c{H}")
                        tc_i = nc.vector.tensor_scalar(
                            tanh_c[:], vph(pts[1], H), 2.0, -1.0, Mult, Add
                        )
                        t2 = gp.tile([128, 2, BLOC], F32, tag=f"t2{H}")
                        t2_i = nc.vector.tensor_tensor(
                            t2[:], vph(pts[0], H), ctT_prev[:, hs, :], Mult
                        )
                        if H == 1:
                            # keep the H=0 critical chain uninterrupted on the
                            # DVE queue: the next step's first gate matmuls are
                            # gated on ht half 0
                            add_dep_helper(tc_i.ins, ht_i.ins, reason="h0 first")
                            add_dep_helper(t2_i.ins, ht_i.ins, reason="h0 first")
                        t1 = gp.tile([128, 2, BLOC], BF16, tag=f"t1{H}")
                        nc.vector.tensor_tensor(t1[:], vph(pts[2], H), tanh_c[:], Mult)
                        nc.vector.tensor_tensor(
                            ctT_new[:, hs, :], t1[:], t2[:], Add
                        )
                        tanh_ct = gp.tile([128, 2, BLOC], BF16, tag=f"tanh_ct{H}")
                        nc.scalar.activation(tanh_ct[:], ctT_new[:, hs, :], Tanh)
                        ht_i = nc.vector.tensor_tensor(
                            ht_hist[:, hs, t + 1, :],
                            vph(pts[3], H),
                            tanh_ct[:],
                            Mult,
                        )
                    if t + 1 < t_steps:
                        pg = pg_next

            # ---------------- phase 3: ys ----------------
            with tc.tile_pool(name="p3sb", bufs=3) as p3, \
                 tc.tile_pool(name="p3ps", bufs=3, space="PSUM") as p3p:
                NT = 512
                n_nt = (tb + NT - 1) // NT
                for m in range(4):
                    for nt in range(n_nt):
                        cols = min(NT, tb - nt * NT)
                        py = p3p.tile([128, NT], F32, tag="py")
                        for k in range(4):
                            nc.tensor.matmul(
                                py[:, :cols],
                                wymT_sb[:, k, m * 128 : (m + 1) * 128],
                                ht_hist[:, k, :, :].rearrange("p t b -> p (t b)")[
                                    :, BLOC + nt * NT : BLOC + nt * NT + cols
                                ],
                                start=(k == 0),
                                stop=(k == 3),
                            )
                        ytile = p3.tile([128, NT], F32, tag="ytile")
                        nc.vector.tensor_copy(ytile[:, :cols], py[:, :cols])
                        nc.sync.dma_start(
                            ysT_d.rearrange("(mm p) n -> mm p n", p=128)[
                                m, :, nt * NT : nt * NT + cols
                            ],
                            ytile[:, :cols],
                        )

    if fix_waits:
        _fix_excess_waits(nc)
    return nc


def _get_nc(t_steps=T, fix_waits=True, split_k3=True):
    key = (t_steps, fix_waits, split_k3)
    if key not in _BUILT:
        _apply_tile_patches()
        _BUILT[key] = _build(t_steps, fix_waits, split_k3)
    return _BUILT[key]


# ---------------------------------------------------------------------------
# Host entry point
# ---------------------------------------------------------------------------
def _prep_in_maps(x, vector_ih, vector_hh, bias_ih, wym_w, indx_ih, indx_hh):
    x = np.asarray(x, dtype=np.float32)
    vector_ih = np.asarray(vector_ih, dtype=np.float32)
    vector_hh = np.asarray(vector_hh, dtype=np.float32)
    bias_ih = np.asarray(bias_ih, dtype=np.float32)
    wym_w = np.asarray(wym_w, dtype=np.float32)
    indx_ih = np.asarray(indx_ih)
    indx_hh = np.asarray(indx_hh)

    t_steps = x.shape[0]
    tb = t_steps * BLOC

    # reconstruct weights (host-side layout prep)
    wihT = vector_ih[indx_ih.reshape(-1).astype(np.int64)].reshape(D, G)
    whh = vector_hh[indx_hh.reshape(-1).astype(np.int64)].reshape(P_DIM, G)
    wc = (wym_w.T.astype(np.float64) @ whh.astype(np.float64)).astype(np.float32)

    wihT = np.ascontiguousarray(wihT[:, _GATE_PERM])
    wc = np.ascontiguousarray(wc[:, _GATE_PERM])
    bias = np.ascontiguousarray(bias_ih[_GATE_PERM]).reshape(1, G).copy()
    # c block (block 1 in [f, c, i, o]) pre-scaled by 2: tanh(x)=2*sig(2x)-1
    wihT[:, 512:1024] *= 2.0
    wc[:, 512:1024] *= 2.0
    bias[:, 512:1024] *= 2.0
    wymT = np.ascontiguousarray(wym_w.T)

    ident = np.eye(BLOC).astype(np.float32)
    identt = np.zeros((128, BLOC), np.float32)
    for g in range(4):
        identt[32 * g : 32 * g + BLOC, 0:BLOC] = np.eye(BLOC)

    in_maps = []
    for i in range(NCORES):
        x_loc = x[:, BLOC * i : BLOC * (i + 1), :].reshape(tb, D)
        xT = np.ascontiguousarray(x_loc.T)
        in_maps.append({
            "xT": xT,
            "wihT": wihT,
            "wc": wc,
            "wymT": wymT,
            "bias": bias,
            "ident": ident,
            "identt": identt,
        })
    return in_maps


def kernel(x, vector_ih, vector_hh, bias_ih, wym_w, indx_ih, indx_hh):
    from concourse.bass_utils import run_bass_kernel_spmd

    in_maps = _prep_in_maps(
        x, vector_ih, vector_hh, bias_ih, wym_w, indx_ih, indx_hh
    )
    t_steps = np.asarray(x).shape[0]
    nc = _get_nc(t_steps)
    res = run_bass_kernel_spmd(nc, in_maps, core_ids=list(range(NCORES)))
    globals()["_LAST_RES"] = res

    out = np.empty((t_steps, B, P_DIM), dtype=np.float32)
    for i in range(NCORES):
        ysT = res.results[i]["ysT"]
        ys_loc = ysT.T.reshape(t_steps, BLOC, P_DIM)
        out[:, BLOC * i : BLOC * (i + 1), :] = ys_loc
    return out
